# revision 1
# baseline (speedup 1.0000x reference)
"""Trainium2 Bass kernel for CompoundEmbedding (embedding-bag sum).

Problem: indices (16384, 50) int32 -> gather rows of weight (100001, 128) f32,
sum over the bag dim -> output (16384, 128) f32.

Strategy: replicate the 51MB table to all 8 NeuronCores, shard the batch
(2048 rows per core). Per core, 16 blocks of 128 batch rows:
  - DMA the block's [128, 50] int32 indices into SBUF,
  - 50 indirect (gather) DMAs, one per bag position l: offsets idx[:, l]
    pull 128 rows (512B each) into columns [l*128,(l+1)*128) of a
    [128, 6400] SBUF tile (partition p = batch row p of the block),
  - pairwise-tree reduce (7 DVE tensor_adds, contiguous slices) -> [128, 128],
  - DMA the block result to DRAM.
The gather rate is bound by SWDGE descriptor generation — measured 1.51us per
128-row indirect DMA (~11.8ns/row) steady-state — so the index loads, the DVE
reduction tree, and the output stores all hide behind it completely:
800 gathers/core * 1.51us ~= 1.21ms ~= the measured end-to-end body time.
Alternatives measured and rejected: dma_gather (10.5ns/row but int16-index
vocab windows force ~8% padding => wash), gpsimd ap_gather from an
SBUF-resident vocab shard (27.8ns/idx, RD_CMD-latency-bound), PE one-hot
matmul (needs a 410MB/core selection-matrix stream).
All shapes/sharding are hardcoded for this problem instance.
"""

import numpy as np

NUM_EMB = 100001
D = 128
B = 16384
BAG = 50
NCORES = 8
P = 128
ROWS_PER_CORE = B // NCORES  # 2048
NBLK = ROWS_PER_CORE // P  # 16

_CACHE = {}


def _build(nblk=NBLK, repeats=1, loop_k=1):
    """Build + compile the SPMD program. `loop_k` wraps the body in a
    hardware For_i loop (used by the timing harness to amortize launch
    overhead); `repeats` replays it unrolled."""
    import contextlib
    import concourse.bass as bass
    import concourse.tile as tile
    from concourse import bacc, mybir

    rows = nblk * P
    nc = bacc.Bacc("TRN2", target_bir_lowering=False, debug=False,
                   num_devices=NCORES)
    idx_d = nc.dram_tensor("idx", [rows, BAG], mybir.dt.int32,
                           kind="ExternalInput").ap()
    w_d = nc.dram_tensor("weight", [NUM_EMB, D], mybir.dt.float32,
                         kind="ExternalInput").ap()
    out_d = nc.dram_tensor("out", [rows, D], mybir.dt.float32,
                           kind="ExternalOutput").ap()

    with tile.TileContext(nc) as tc:
        with tc.tile_pool(name="idxp", bufs=2) as idxp, \
             tc.tile_pool(name="gat", bufs=2) as gatp, \
             tc.tile_pool(name="red", bufs=2) as redp:
            loop_stack = contextlib.ExitStack()
            if loop_k > 1:
                loop_stack.enter_context(tc.For_i(0, loop_k, 1))
                nc.tensor.nop()
                nc.scalar.nop()
            for _rep in range(repeats):
                for blk in range(nblk):
                    it = idxp.tile([P, BAG], mybir.dt.int32)
                    nc.sync.dma_start(out=it[:],
                                      in_=idx_d[blk * P:(blk + 1) * P, :])
                    gt = gatp.tile([P, BAG * D], mybir.dt.float32)
                    for l in range(BAG):
                        nc.gpsimd.indirect_dma_start(
                            out=gt[:, l * D:(l + 1) * D],
                            out_offset=None,
                            in_=w_d[:],
                            in_offset=bass.IndirectOffsetOnAxis(
                                ap=it[:, l:l + 1], axis=0),
                        )
                    # tree-reduce the 50 rows per partition down to 1
                    r25 = redp.tile([P, 25 * D], mybir.dt.float32)
                    nc.vector.tensor_add(r25[:], gt[:, 0:25 * D],
                                         gt[:, 25 * D:50 * D])
                    r12 = redp.tile([P, 12 * D], mybir.dt.float32)
                    nc.vector.tensor_add(r12[:], r25[:, 0:12 * D],
                                         r25[:, 12 * D:24 * D])
                    r6 = redp.tile([P, 6 * D], mybir.dt.float32)
                    nc.vector.tensor_add(r6[:], r12[:, 0:6 * D],
                                         r12[:, 6 * D:12 * D])
                    r3 = redp.tile([P, 3 * D], mybir.dt.float32)
                    nc.vector.tensor_add(r3[:], r6[:, 0:3 * D],
                                         r6[:, 3 * D:6 * D])
                    r1 = redp.tile([P, D], mybir.dt.float32)
                    nc.vector.tensor_add(r1[:], r3[:, 0:D], r3[:, D:2 * D])
                    r1b = redp.tile([P, D], mybir.dt.float32)
                    nc.vector.tensor_add(r1b[:], r3[:, 2 * D:3 * D],
                                         r25[:, 24 * D:25 * D])
                    rf = redp.tile([P, D], mybir.dt.float32)
                    nc.vector.tensor_add(rf[:], r1[:], r1b[:])
                    nc.sync.dma_start(out=out_d[blk * P:(blk + 1) * P, :],
                                      in_=rf[:])
            loop_stack.close()
    nc.compile()
    return nc


def _get_program(nblk=NBLK, repeats=1, loop_k=1):
    key = (nblk, repeats, loop_k)
    if key not in _CACHE:
        _CACHE[key] = _build(nblk, repeats, loop_k)
    return _CACHE[key]


def kernel(input, weight):
    from concourse.bass_utils import run_bass_kernel_spmd

    idx = np.ascontiguousarray(np.asarray(input).astype(np.int32))
    w = np.ascontiguousarray(np.asarray(weight, dtype=np.float32))
    assert idx.shape == (B, BAG) and w.shape == (NUM_EMB, D)

    nc = _get_program()
    in_maps = [
        {"idx": idx[c * ROWS_PER_CORE:(c + 1) * ROWS_PER_CORE], "weight": w}
        for c in range(NCORES)
    ]
    res = run_bass_kernel_spmd(nc, in_maps, core_ids=list(range(NCORES)))
    out = np.concatenate([res.results[c]["out"] for c in range(NCORES)], axis=0)
    return out

